# revision 13
# baseline (speedup 1.0000x reference)
"""KAN layer (pykan KANLayer forward) as a Trainium2 Bass kernel.

Math: uniform grid (linspace(-1,1,6), h=0.4) makes every cubic B-spline
cardinal, so the layer collapses to a feature map + accumulated matmuls:

    out[b,o] = sum_m W_m[i,o] relu(t_i - m)^3 + sum_i A[i,o] silu(x_i),
    t = u + 5.5,  u = x/h.

Key restructuring vs the plain relu^3 ladder: planes m<=5 are *leftified*
with relu(s)^3 = s^3 + relu(-s)^3.  Their polynomial parts fold (exactly,
on host, in float64) into one global cubic P3(u) evaluated from features
{u, u^2, u^3} shared across planes, leaving only small bounded residuals
relu(m-t)^3 = -(min(u-(m-5.5),0))^3.  Residual planes {0,1} and right
planes {10,11} are negligible (checked: total rel err 6.9e-4 in f16).
All features fit f16; no f32 chains remain.  The constant poly term is a
per-partition f32 bias folded into the PSUM->SBUF copy (ACT Identity).

Kept planes: left residuals {2,3,4,5} (min-chains, sign folded into
weights), right {6,7,8,9} (max-chains); each pair shares one immediate on
the [u; u-1] input tile.  Poly squares/cubes reuse the full X2 tile
([u^2;(u-1)^2], [u^3;(u-1)^3]) with zero weights on the (u-1) halves.

ACT-parallel squares: Square(u+c) (ACT, bias=c) equals relu(u+c)^2 where
relu(u+c)>0 and is masked by the relu factor elsewhere, so cube =
Square(u+c) * relu(u+c) and the square needs no TS dependency.

Output: PSUM -> SBUF f16 (ACT/DVE halves, +bias), then a *prepared*
SWDGE dma_scatter_add (identity indices, zeroed DRAM output) triggered
from Pool: the descriptor generation runs during compute, so the tail is
just trigger + transfer + completion sem.  The auto epilogue barrier is
deleted; SP waits the DMA completion sem.

Sharding: data-parallel over batch (8 cores x 256 rows).
"""

import numpy as np

B_TOTAL, IN_DIM, OUT_DIM = 2048, 64, 64
N_CORES = 8
B_SH = B_TOTAL // N_CORES  # 256 batch rows per core

_STATE = {}


def _fold(grid, coef, scale_base, scale_sp, mask):
    """Fold spline coefs + scales + mask into per-plane matmul weights."""
    g0 = np.float64(grid[0, 0])
    h = (np.float64(grid[0, -1]) - g0) / (grid.shape[1] - 1)
    inv_h = 1.0 / h
    ctr = 3.0 - g0 * inv_h  # t = u + ctr

    C = (mask * scale_sp)[:, None].astype(np.float64) * coef.astype(np.float64)
    C = C.reshape(OUT_DIM, IN_DIM, 8)
    st = np.array([1.0, -4.0, 6.0, -4.0, 1.0], np.float64) / 6.0
    Wm = np.zeros((12, IN_DIM, OUT_DIM), np.float64)
    for m in range(12):
        for j in range(max(0, m - 4), min(8, m + 1)):
            Wm[m] += C[:, :, j].T * st[m - j]
    A = (mask * scale_base).astype(np.float64).reshape(OUT_DIM, IN_DIM).T
    return Wm, A, float(h), float(inv_h), float(ctr)


def _host_prep(inputs, grid, coef, scale_base, scale_sp, mask):
    Wm, A, h, inv_h, ctr = _fold(grid, coef, scale_base, scale_sp, mask)

    # poly coefficients over leftified planes m in {0..5}
    L = range(6)
    a3 = np.zeros((IN_DIM, OUT_DIM)); a2 = np.zeros_like(a3)
    a1 = np.zeros_like(a3); a0 = np.zeros_like(a3)
    for m in L:
        d = ctr - m
        a3 += Wm[m]; a2 += 3 * d * Wm[m]
        a1 += 3 * d * d * Wm[m]; a0 += d ** 3 * Wm[m]
    bias = a0.sum(axis=0)  # (64,) f32 per-o constant

    # weight tile f16 (128, 8*64); blocks in matmul emission order:
    #  0: X2-linear [a1; 0]   1: silu A (64p)   2: SQ [a2; 0]
    #  3: CU [a3; 0]          4: chain A [-W2; -W3]  5: chain B [-W4; -W5]
    #  6: chain C [W6; W7]    7: chain D [W8; W9]
    wh = np.zeros((128, 8 * OUT_DIM), np.float64)
    wh[0:64, 0 * 64:1 * 64] = a1
    wh[0:64, 1 * 64:2 * 64] = A
    wh[0:64, 2 * 64:3 * 64] = a2
    wh[0:64, 3 * 64:4 * 64] = a3
    for k, (m0, sgn) in enumerate(((2, -1.0), (4, -1.0), (6, 1.0), (8, 1.0))):
        wh[0:64, (4 + k) * 64:(5 + k) * 64] = sgn * Wm[m0]
        wh[64:128, (4 + k) * 64:(5 + k) * 64] = sgn * Wm[m0 + 1]

    # input tile per core: [u; u-1]
    x = inputs.astype(np.float64)
    u_full = (x * inv_h).T  # (64, 2048)
    xs = []
    for c in range(N_CORES):
        u = u_full[:, c * B_SH:(c + 1) * B_SH]
        x2 = np.zeros((128, B_SH), np.float64)
        x2[0:64] = u
        x2[64:128] = u - 1.0
        xs.append(np.ascontiguousarray(x2.astype(np.float16)))

    return (xs, np.ascontiguousarray(wh.astype(np.float16)),
            np.ascontiguousarray(bias[:, None].astype(np.float32)),
            h, ctr)


def _build_nc(h=0.4, ctr=5.5):
    import concourse.bass as bass
    import concourse.bacc as bacc
    import concourse.mybir as mybir
    import concourse.tile as tile

    f32 = mybir.dt.float32
    f16 = mybir.dt.float16
    i16 = mybir.dt.int16
    AF = mybir.ActivationFunctionType
    ALU = mybir.AluOpType

    nc = bacc.Bacc("TRN2", target_bir_lowering=False, debug=False,
                   num_devices=N_CORES)
    xt = nc.dram_tensor("xt", [128, B_SH], f16, kind="ExternalInput")
    whd = nc.dram_tensor("wh", [128, 8 * OUT_DIM], f16, kind="ExternalInput")
    bsd = nc.dram_tensor("bs", [OUT_DIM, 1], f32, kind="ExternalInput")
    # 128 scatter tokens (ucode processes round_up(num_idxs,128)); rows
    # 64..127 receive garbage from the unwritten O top half - host ignores.
    out = nc.dram_tensor("out", [128, B_SH], f16, kind="ExternalOutput")

    # chain immediates on the [u; u-1] tile
    cA, cB = 2 - ctr, 4 - ctr            # min-chains: min(u - c, 0)
    iC, iD = ctr - 6, ctr - 8            # max-chains: max(u + i, 0)

    # const APs for the ACT Square biases (registered like Bacc's init
    # consts: memset pre-barrier on the otherwise-idle Pool engine)
    for v in (float(-cA), float(-cB), float(iC)):
        t = nc.alloc_sbuf_tensor(f"const-float32-{v}", [128, 1],
                                 mybir.dt.float32)
        nc.gpsimd.memset(t.ap(), v)
        nc.const_aps.aps[(mybir.dt.float32, v)] = t.ap()

    with tile.TileContext(nc) as tc:
        with tc.tile_pool(name="c", bufs=1) as cp, \
             tc.tile_pool(name="ps", bufs=1, space=bass.MemorySpace.PSUM) as pp:
            X2 = cp.tile([128, B_SH], f16)
            WH = cp.tile([128, 8 * OUT_DIM], f16)
            BS = cp.tile([OUT_DIM, 1], f32)
            nc.sync.dma_start(X2[:], xt[:])
            nc.sync.dma_start(WH[:], whd[:])
            nc.scalar.dma_start(BS[:], bsd[:])

            psum = pp.tile([OUT_DIM, B_SH], f32)

            # scatter-out bookkeeping: identity idxs via iota (the prep is
            # emitted after the O writes so its deferred data deps land on
            # the trigger; with no sem waits it still runs early on Pool)
            IDX = cp.tile([128, 8], i16)
            nc.gpsimd.iota(IDX[:], [[16, 8]], channel_multiplier=1)
            O = cp.tile([128, 1, B_SH], f16)
            dma_sem = nc.alloc_semaphore("out_dma")
            # prep emitted EARLY (descriptor gen runs on the idle Pool engine
            # at ~1900-2950, far from the trigger); because O is unwritten at
            # emission the deferred data deps are empty, so the trigger below
            # carries a manual sync dep on the copy op.
            nc.gpsimd.dma_scatter_add(
                out[:], O[:], IDX[:], num_idxs=128, num_idxs_reg=128,
                elem_size=B_SH, prepare_only=True, sem=dma_sem)

            # ACT: parallel squares for chains A,B (Square(x + c) masked by
            # the relu/min factor later), then silu.
            SpA = cp.tile([128, B_SH], f16)
            nc.scalar.activation(SpA[:], X2[:], AF.Square, bias=float(-cA))
            SpB = cp.tile([128, B_SH], f16)
            nc.scalar.activation(SpB[:], X2[:], AF.Square, bias=float(-cB))
            SpC = cp.tile([128, B_SH], f16)
            nc.scalar.activation(SpC[:], X2[:], AF.Square, bias=float(iC))
            SIL = cp.tile([64, B_SH], f16)
            nc.scalar.activation(SIL[:], X2[0:64, :], AF.Silu, scale=h)

            # DVE: TS shifts; TS_D first so the Pool chain starts early
            RD = cp.tile([128, B_SH], f16)
            nc.vector.tensor_scalar(RD[:], X2[:], iD, 0.0, ALU.add, ALU.max)
            RA = cp.tile([128, B_SH], f16)
            nc.vector.tensor_scalar(RA[:], X2[:], cA, 0.0, ALU.subtract, ALU.min)
            RB = cp.tile([128, B_SH], f16)
            nc.vector.tensor_scalar(RB[:], X2[:], cB, 0.0, ALU.subtract, ALU.min)
            RC = cp.tile([128, B_SH], f16)
            nc.vector.tensor_scalar(RC[:], X2[:], iC, 0.0, ALU.add, ALU.max)

            # Pool: square of chain D (after TS_D), cube of D later
            SD = cp.tile([128, B_SH], f16)
            nc.gpsimd.tensor_mul(SD[:], RD[:], RD[:])

            # DVE: poly square+cube first (only need X2), then chains;
            # nosync-chained so the tile scheduler keeps this order
            HB = B_SH // 2
            SQ = cp.tile([128, B_SH], f16)
            i1 = nc.vector.tensor_mul(SQ[:], X2[:], X2[:])
            CU = cp.tile([128, B_SH], f16)
            i2 = nc.vector.tensor_mul(CU[:], SQ[:], X2[:])
            CA = cp.tile([128, B_SH], f16)
            i3 = nc.vector.tensor_mul(CA[:], SpA[:], RA[:])
            # chain D cube split: Pool half + DVE half (frees Pool early
            # so the scatter prep never gates the trigger)
            CD = cp.tile([128, B_SH], f16)
            nc.gpsimd.tensor_mul(CD[:, 0:HB], SD[:, 0:HB], RD[:, 0:HB])
            i4 = nc.vector.tensor_mul(CD[:, HB:], SD[:, HB:], RD[:, HB:])
            CC = cp.tile([128, B_SH], f16)
            i5 = nc.vector.tensor_mul(CC[:], SpC[:], RC[:])
            CB = cp.tile([128, B_SH], f16)
            i6 = nc.vector.tensor_mul(CB[:], SpB[:], RB[:])
            from concourse.instruction_name_ordered_set import \
                InstructionNameOrderedSet
            for a, b in zip((i1, i2, i3, i4, i5), (i2, i3, i4, i5, i6)):
                _s = InstructionNameOrderedSet()
                _s.add(a.ins.name)
                b.ins.add_nosync_dependencies_from(_s)

            # matmuls in expected readiness order
            nc.tensor.matmul(psum[:], WH[:, 0 * 64:1 * 64], X2[:],
                             start=True, stop=False)
            nc.tensor.matmul(psum[:], WH[:, 2 * 64:3 * 64], SQ[:],
                             start=False, stop=False)
            nc.tensor.matmul(psum[:], WH[:, 3 * 64:4 * 64], CU[:],
                             start=False, stop=False)
            nc.tensor.matmul(psum[:], WH[:, 4 * 64:5 * 64], CA[:],
                             start=False, stop=False)
            nc.tensor.matmul(psum[:], WH[:, 7 * 64:8 * 64], CD[:],
                             start=False, stop=False)
            nc.tensor.matmul(psum[:], WH[:, 6 * 64:7 * 64], CC[:],
                             start=False, stop=False)
            nc.tensor.matmul(psum[:], WH[:, 5 * 64:6 * 64], CB[:],
                             start=False, stop=False)
            nc.tensor.matmul(psum[:], WH[0:64, 1 * 64:2 * 64], SIL[:],
                             start=False, stop=True)

            # PSUM -> SBUF f16 with per-o bias (single full-width ACT op)
            cpy = nc.scalar.activation(O[0:64, 0, :], psum[:],
                                       AF.Identity, bias=BS[:])

            trig = nc.gpsimd.trigger_dma(count=None)
            _d = InstructionNameOrderedSet()
            _d.add(cpy.ins.name)
            trig.ins.add_sync_dependencies_from(_d)
            nc.sync.wait_ge(dma_sem, 16)

    _hoist_input_dmas(nc, mybir)

    # Emit the activation-table load (silu_and_others, set 18) before the
    # init barrier so it overlaps the input-DMA latency.
    atl = mybir.InstLoadActFuncSet(name=nc.get_next_instruction_name(),
                                   act_func_set_id=18, ins=[], outs=[])
    atl.engine = mybir.EngineType.Activation
    main = nc.main_func.blocks[0]
    pos = next(k for k, i in enumerate(main.instructions)
               if isinstance(i, mybir.InstDrain)
               and i.engine == mybir.EngineType.Activation)
    main.instructions.insert(pos, atl)

    # Delete the auto epilogue barrier entirely: the only end-of-program
    # obligation is the out-DMA completion sem, waited on SP inside the
    # tile block.
    epi = nc.main_func.blocks[2]
    del epi.instructions[:]

    nc.compile()
    return nc


def _hoist_input_dmas(nc, mybir):
    """Move the X2/WH input DMAs ahead of the init all-engine barrier."""
    main = nc.main_func.blocks[0]
    tileblk = nc.main_func.blocks[1]

    sp_dmas = [i for i in tileblk.instructions
               if isinstance(i, mybir.InstDMACopy)
               and i.engine == mybir.EngineType.SP][:2]   # xt, wh loads

    sp_drain = next(k for k, i in enumerate(main.instructions)
                    if isinstance(i, mybir.InstDrain)
                    and i.engine == mybir.EngineType.SP)
    for insn in reversed(sp_dmas):
        tileblk.instructions.remove(insn)
        main.instructions.insert(sp_drain, insn)


def kernel(**inputs):
    x = np.asarray(inputs["inputs"], dtype=np.float32)
    grid = np.asarray(inputs["grid"], dtype=np.float32)
    coef = np.asarray(inputs["coef"], dtype=np.float32)
    scale_base = np.asarray(inputs["scale_base"], dtype=np.float32)
    scale_sp = np.asarray(inputs["scale_sp"], dtype=np.float32)
    mask = np.asarray(inputs["mask"], dtype=np.float32)

    xs, wh, bs, h, ctr = _host_prep(x, grid, coef, scale_base, scale_sp, mask)

    key = ("nc", h, ctr)
    if key not in _STATE:
        _STATE[key] = _build_nc(h, ctr)
    nc = _STATE[key]

    from concourse.bass_utils import run_bass_kernel_spmd

    in_maps = [{"xt": xs[c], "wh": wh, "bs": bs} for c in range(N_CORES)]

    res = run_bass_kernel_spmd(nc, in_maps, list(range(N_CORES)),
                               **_STATE.get("run_kwargs", {}))
    _STATE["last_results"] = res
    out_t = np.concatenate([res.results[c]["out"][0:OUT_DIM]
                            for c in range(N_CORES)], axis=1)  # (64, 2048)
    return np.ascontiguousarray(out_t.T).astype(np.float32)


# revision 14
# speedup vs baseline: 1.0134x; 1.0134x over previous
"""KAN layer (pykan KANLayer forward) as a Trainium2 Bass kernel.

Math: uniform grid (linspace(-1,1,6), h=0.4) makes every cubic B-spline
cardinal, so the layer collapses to a feature map + accumulated matmuls:

    out[b,o] = sum_m W_m[i,o] relu(t_i - m)^3 + sum_i A[i,o] silu(x_i),
    t = u + 5.5,  u = x/h.

Key restructuring vs the plain relu^3 ladder: planes m<=5 are *leftified*
with relu(s)^3 = s^3 + relu(-s)^3.  Their polynomial parts fold (exactly,
on host, in float64) into one global cubic P3(u) evaluated from features
{u, u^2, u^3} shared across planes, leaving only small bounded residuals
relu(m-t)^3 = -(min(u-(m-5.5),0))^3.  Residual planes {0,1} and right
planes {10,11} are negligible (checked: total rel err 6.9e-4 in f16).
All features fit f16; no f32 chains remain.  The constant poly term is a
per-partition f32 bias folded into the PSUM->SBUF copy (ACT Identity).

Kept planes: left residuals {2,3,4,5} (min-chains, sign folded into
weights), right {6,7,8,9} (max-chains); each pair shares one immediate on
the [u; u-1] input tile.  Poly squares/cubes reuse the full X2 tile
([u^2;(u-1)^2], [u^3;(u-1)^3]) with zero weights on the (u-1) halves.

ACT-parallel squares: Square(u+c) (ACT, bias=c) equals relu(u+c)^2 where
relu(u+c)>0 and is masked by the relu factor elsewhere, so cube =
Square(u+c) * relu(u+c) and the square needs no TS dependency.

Output: PSUM -> SBUF f16 (ACT/DVE halves, +bias), then a *prepared*
SWDGE dma_scatter_add (identity indices, zeroed DRAM output) triggered
from Pool: the descriptor generation runs during compute, so the tail is
just trigger + transfer + completion sem.  The auto epilogue barrier is
deleted; SP waits the DMA completion sem.

Sharding: data-parallel over batch (8 cores x 256 rows).
"""

import numpy as np

B_TOTAL, IN_DIM, OUT_DIM = 2048, 64, 64
N_CORES = 8
B_SH = B_TOTAL // N_CORES  # 256 batch rows per core

_STATE = {}


def _fold(grid, coef, scale_base, scale_sp, mask):
    """Fold spline coefs + scales + mask into per-plane matmul weights."""
    g0 = np.float64(grid[0, 0])
    h = (np.float64(grid[0, -1]) - g0) / (grid.shape[1] - 1)
    inv_h = 1.0 / h
    ctr = 3.0 - g0 * inv_h  # t = u + ctr

    C = (mask * scale_sp)[:, None].astype(np.float64) * coef.astype(np.float64)
    C = C.reshape(OUT_DIM, IN_DIM, 8)
    st = np.array([1.0, -4.0, 6.0, -4.0, 1.0], np.float64) / 6.0
    Wm = np.zeros((12, IN_DIM, OUT_DIM), np.float64)
    for m in range(12):
        for j in range(max(0, m - 4), min(8, m + 1)):
            Wm[m] += C[:, :, j].T * st[m - j]
    A = (mask * scale_base).astype(np.float64).reshape(OUT_DIM, IN_DIM).T
    return Wm, A, float(h), float(inv_h), float(ctr)


def _host_prep(inputs, grid, coef, scale_base, scale_sp, mask):
    Wm, A, h, inv_h, ctr = _fold(grid, coef, scale_base, scale_sp, mask)

    # poly coefficients over leftified planes m in {0..5}
    L = range(6)
    a3 = np.zeros((IN_DIM, OUT_DIM)); a2 = np.zeros_like(a3)
    a1 = np.zeros_like(a3); a0 = np.zeros_like(a3)
    for m in L:
        d = ctr - m
        a3 += Wm[m]; a2 += 3 * d * Wm[m]
        a1 += 3 * d * d * Wm[m]; a0 += d ** 3 * Wm[m]
    bias = a0.sum(axis=0)  # (64,) f32 per-o constant

    # weight tile f16 (128, 8*64); blocks in matmul emission order:
    #  0: X2-linear [a1; 0]   1: silu A (64p)   2: SQ [a2; 0]
    #  3: CU [a3; 0]          4: chain A [-W2; -W3]  5: chain B [-W4; -W5]
    #  6: chain C [W6; W7]    7: chain D [W8; W9]
    wh = np.zeros((128, 8 * OUT_DIM), np.float64)
    wh[0:64, 0 * 64:1 * 64] = a1
    wh[0:64, 1 * 64:2 * 64] = A
    wh[0:64, 2 * 64:3 * 64] = a2
    wh[0:64, 3 * 64:4 * 64] = a3
    for k, (m0, sgn) in enumerate(((2, -1.0), (4, -1.0), (6, 1.0), (8, 1.0))):
        wh[0:64, (4 + k) * 64:(5 + k) * 64] = sgn * Wm[m0]
        wh[64:128, (4 + k) * 64:(5 + k) * 64] = sgn * Wm[m0 + 1]

    # input tile per core: [u; u-1]
    x = inputs.astype(np.float64)
    u_full = (x * inv_h).T  # (64, 2048)
    xs = []
    for c in range(N_CORES):
        u = u_full[:, c * B_SH:(c + 1) * B_SH]
        x2 = np.zeros((128, B_SH), np.float64)
        x2[0:64] = u
        x2[64:128] = u - 1.0
        xs.append(np.ascontiguousarray(x2.astype(np.float16)))

    return (xs, np.ascontiguousarray(wh.astype(np.float16)),
            np.ascontiguousarray(bias[:, None].astype(np.float32)),
            h, ctr)


def _build_nc(h=0.4, ctr=5.5):
    import concourse.bass as bass
    import concourse.bacc as bacc
    import concourse.mybir as mybir
    import concourse.tile as tile

    f32 = mybir.dt.float32
    f16 = mybir.dt.float16
    i16 = mybir.dt.int16
    AF = mybir.ActivationFunctionType
    ALU = mybir.AluOpType

    nc = bacc.Bacc("TRN2", target_bir_lowering=False, debug=False,
                   num_devices=N_CORES)
    xt = nc.dram_tensor("xt", [128, B_SH], f16, kind="ExternalInput")
    whd = nc.dram_tensor("wh", [128, 8 * OUT_DIM], f16, kind="ExternalInput")
    bsd = nc.dram_tensor("bs", [OUT_DIM, 1], f32, kind="ExternalInput")
    # 128 scatter tokens (ucode processes round_up(num_idxs,128)); rows
    # 64..127 receive garbage from the unwritten O top half - host ignores.
    out = nc.dram_tensor("out", [128, B_SH], f16, kind="ExternalOutput")

    # chain immediates on the [u; u-1] tile
    cA, cB = 2 - ctr, 4 - ctr            # min-chains: min(u - c, 0)
    iC, iD = ctr - 6, ctr - 8            # max-chains: max(u + i, 0)

    # const APs for the ACT Square biases (registered like Bacc's init
    # consts: memset pre-barrier on the otherwise-idle Pool engine)
    for v in (float(-cA), float(-cB), float(iC)):
        t = nc.alloc_sbuf_tensor(f"const-float32-{v}", [128, 1],
                                 mybir.dt.float32)
        nc.gpsimd.memset(t.ap(), v)
        nc.const_aps.aps[(mybir.dt.float32, v)] = t.ap()

    with tile.TileContext(nc) as tc:
        with tc.tile_pool(name="c", bufs=1) as cp, \
             tc.tile_pool(name="ps", bufs=1, space=bass.MemorySpace.PSUM) as pp:
            X2 = cp.tile([128, B_SH], f16)
            WH = cp.tile([128, 8 * OUT_DIM], f16)
            BS = cp.tile([OUT_DIM, 1], f32)
            nc.sync.dma_start(X2[:], xt[:])
            nc.sync.dma_start(WH[:], whd[:])
            nc.scalar.dma_start(BS[:], bsd[:])

            psum = pp.tile([OUT_DIM, B_SH], f32)

            # scatter-out bookkeeping: identity idxs via iota (the prep is
            # emitted after the O writes so its deferred data deps land on
            # the trigger; with no sem waits it still runs early on Pool)
            IDX = cp.tile([128, 8], i16)
            nc.gpsimd.iota(IDX[:], [[16, 8]], channel_multiplier=1)
            O = cp.tile([128, 1, B_SH], f16)
            dma_sem = nc.alloc_semaphore("out_dma")
            # prep emitted EARLY (descriptor gen runs on the idle Pool engine
            # at ~1900-2950, far from the trigger); because O is unwritten at
            # emission the deferred data deps are empty, so the trigger below
            # carries a manual sync dep on the copy op.
            nc.gpsimd.dma_scatter_add(
                out[:], O[:], IDX[:], num_idxs=128, num_idxs_reg=128,
                elem_size=B_SH, prepare_only=True, sem=dma_sem)

            # ACT: parallel squares for chains A,B (Square(x + c) masked by
            # the relu/min factor later), then silu.
            SpA = cp.tile([128, B_SH], f16)
            nc.scalar.activation(SpA[:], X2[:], AF.Square, bias=float(-cA))
            SpB = cp.tile([128, B_SH], f16)
            nc.scalar.activation(SpB[:], X2[:], AF.Square, bias=float(-cB))
            SpC = cp.tile([128, B_SH], f16)
            nc.scalar.activation(SpC[:], X2[:], AF.Square, bias=float(iC))
            SIL = cp.tile([64, B_SH], f16)
            nc.scalar.activation(SIL[:], X2[0:64, :], AF.Silu, scale=h)

            # DVE: TS shifts; TS_D first so the Pool chain starts early
            RD = cp.tile([128, B_SH], f16)
            nc.vector.tensor_scalar(RD[:], X2[:], iD, 0.0, ALU.add, ALU.max)
            RA = cp.tile([128, B_SH], f16)
            nc.vector.tensor_scalar(RA[:], X2[:], cA, 0.0, ALU.subtract, ALU.min)
            RB = cp.tile([128, B_SH], f16)
            nc.vector.tensor_scalar(RB[:], X2[:], cB, 0.0, ALU.subtract, ALU.min)
            RC = cp.tile([128, B_SH], f16)
            nc.vector.tensor_scalar(RC[:], X2[:], iC, 0.0, ALU.add, ALU.max)

            # Pool: square of chain D (after TS_D), cube of D later
            SD = cp.tile([128, B_SH], f16)
            nc.gpsimd.tensor_mul(SD[:], RD[:], RD[:])

            # DVE: poly square+cube first (only need X2), then chains;
            # nosync-chained so the tile scheduler keeps this order
            HB = B_SH // 2
            SQ = cp.tile([128, B_SH], f16)
            i1 = nc.vector.tensor_mul(SQ[:], X2[:], X2[:])
            CA = cp.tile([128, B_SH], f16)
            i2 = nc.vector.tensor_mul(CA[:], SpA[:], RA[:])
            CU = cp.tile([128, B_SH], f16)
            i3 = nc.vector.tensor_mul(CU[:], SQ[:], X2[:])
            # chain D cube split: Pool half + DVE half (frees Pool early
            # so the scatter prep never gates the trigger)
            CD = cp.tile([128, B_SH], f16)
            nc.gpsimd.tensor_mul(CD[:, 0:HB], SD[:, 0:HB], RD[:, 0:HB])
            i4 = nc.vector.tensor_mul(CD[:, HB:], SD[:, HB:], RD[:, HB:])
            CC = cp.tile([128, B_SH], f16)
            i5 = nc.vector.tensor_mul(CC[:], SpC[:], RC[:])
            CB = cp.tile([128, B_SH], f16)
            i6 = nc.vector.tensor_mul(CB[:], SpB[:], RB[:])
            from concourse.instruction_name_ordered_set import \
                InstructionNameOrderedSet
            for a, b in zip((i1, i2, i3, i4, i5), (i2, i3, i4, i5, i6)):
                _s = InstructionNameOrderedSet()
                _s.add(a.ins.name)
                b.ins.add_nosync_dependencies_from(_s)

            # matmuls in expected readiness order
            nc.tensor.matmul(psum[:], WH[:, 0 * 64:1 * 64], X2[:],
                             start=True, stop=False)
            nc.tensor.matmul(psum[:], WH[:, 2 * 64:3 * 64], SQ[:],
                             start=False, stop=False)
            nc.tensor.matmul(psum[:], WH[:, 3 * 64:4 * 64], CU[:],
                             start=False, stop=False)
            nc.tensor.matmul(psum[:], WH[:, 4 * 64:5 * 64], CA[:],
                             start=False, stop=False)
            nc.tensor.matmul(psum[:], WH[:, 7 * 64:8 * 64], CD[:],
                             start=False, stop=False)
            nc.tensor.matmul(psum[:], WH[:, 6 * 64:7 * 64], CC[:],
                             start=False, stop=False)
            nc.tensor.matmul(psum[:], WH[0:64, 1 * 64:2 * 64], SIL[:],
                             start=False, stop=False)
            nc.tensor.matmul(psum[:], WH[:, 5 * 64:6 * 64], CB[:],
                             start=False, stop=True)

            # PSUM -> SBUF f16 with per-o bias (single full-width ACT op)
            cpy = nc.scalar.activation(O[0:64, 0, :], psum[:],
                                       AF.Identity, bias=BS[:])

            trig = nc.gpsimd.trigger_dma(count=None)
            _d = InstructionNameOrderedSet()
            _d.add(cpy.ins.name)
            trig.ins.add_sync_dependencies_from(_d)
            nc.sync.wait_ge(dma_sem, 16)

    _hoist_input_dmas(nc, mybir)

    # Emit the activation-table load (silu_and_others, set 18) before the
    # init barrier so it overlaps the input-DMA latency.
    atl = mybir.InstLoadActFuncSet(name=nc.get_next_instruction_name(),
                                   act_func_set_id=18, ins=[], outs=[])
    atl.engine = mybir.EngineType.Activation
    main = nc.main_func.blocks[0]
    pos = next(k for k, i in enumerate(main.instructions)
               if isinstance(i, mybir.InstDrain)
               and i.engine == mybir.EngineType.Activation)
    main.instructions.insert(pos, atl)

    # Delete the auto epilogue barrier entirely: the only end-of-program
    # obligation is the out-DMA completion sem, waited on SP inside the
    # tile block.
    epi = nc.main_func.blocks[2]
    del epi.instructions[:]

    nc.compile()
    return nc


def _hoist_input_dmas(nc, mybir):
    """Move the X2/WH input DMAs ahead of the init all-engine barrier."""
    main = nc.main_func.blocks[0]
    tileblk = nc.main_func.blocks[1]

    sp_dmas = [i for i in tileblk.instructions
               if isinstance(i, mybir.InstDMACopy)
               and i.engine == mybir.EngineType.SP][:2]   # xt, wh loads

    sp_drain = next(k for k, i in enumerate(main.instructions)
                    if isinstance(i, mybir.InstDrain)
                    and i.engine == mybir.EngineType.SP)
    for insn in reversed(sp_dmas):
        tileblk.instructions.remove(insn)
        main.instructions.insert(sp_drain, insn)


def kernel(**inputs):
    x = np.asarray(inputs["inputs"], dtype=np.float32)
    grid = np.asarray(inputs["grid"], dtype=np.float32)
    coef = np.asarray(inputs["coef"], dtype=np.float32)
    scale_base = np.asarray(inputs["scale_base"], dtype=np.float32)
    scale_sp = np.asarray(inputs["scale_sp"], dtype=np.float32)
    mask = np.asarray(inputs["mask"], dtype=np.float32)

    xs, wh, bs, h, ctr = _host_prep(x, grid, coef, scale_base, scale_sp, mask)

    key = ("nc", h, ctr)
    if key not in _STATE:
        _STATE[key] = _build_nc(h, ctr)
    nc = _STATE[key]

    from concourse.bass_utils import run_bass_kernel_spmd

    in_maps = [{"xt": xs[c], "wh": wh, "bs": bs} for c in range(N_CORES)]

    res = run_bass_kernel_spmd(nc, in_maps, list(range(N_CORES)),
                               **_STATE.get("run_kwargs", {}))
    _STATE["last_results"] = res
    out_t = np.concatenate([res.results[c]["out"][0:OUT_DIM]
                            for c in range(N_CORES)], axis=1)  # (64, 2048)
    return np.ascontiguousarray(out_t.T).astype(np.float32)


# revision 15
# speedup vs baseline: 1.0309x; 1.0172x over previous
"""KAN layer (pykan KANLayer forward) as a Trainium2 Bass kernel.

Math: uniform grid (linspace(-1,1,6), h=0.4) makes every cubic B-spline
cardinal, so the layer collapses to a feature map + accumulated matmuls:

    out[b,o] = sum_m W_m[i,o] relu(t_i - m)^3 + sum_i A[i,o] silu(x_i),
    t = u + 5.5,  u = x/h.

Key restructuring vs the plain relu^3 ladder: planes m<=5 are *leftified*
with relu(s)^3 = s^3 + relu(-s)^3.  Their polynomial parts fold (exactly,
on host, in float64) into one global cubic P3(u) evaluated from features
{u, u^2, u^3} shared across planes, leaving only small bounded residuals
relu(m-t)^3 = -(min(u-(m-5.5),0))^3.  Residual planes {0,1} and right
planes {10,11} are negligible (checked: total rel err 6.9e-4 in f16).
All features fit f16; no f32 chains remain.  The constant poly term is a
per-partition f32 bias folded into the PSUM->SBUF copy (ACT Identity).

Kept planes: left residuals {2,3,4,5} (min-chains, sign folded into
weights), right {6,7,8,9} (max-chains); each pair shares one immediate on
the [u; u-1] input tile.  Poly squares/cubes reuse the full X2 tile
([u^2;(u-1)^2], [u^3;(u-1)^3]) with zero weights on the (u-1) halves.

ACT-parallel squares: Square(u+c) (ACT, bias=c) equals relu(u+c)^2 where
relu(u+c)>0 and is masked by the relu factor elsewhere, so cube =
Square(u+c) * relu(u+c) and the square needs no TS dependency.

Output: PSUM -> SBUF f16 (ACT/DVE halves, +bias), then a *prepared*
SWDGE dma_scatter_add (identity indices, zeroed DRAM output) triggered
from Pool: the descriptor generation runs during compute, so the tail is
just trigger + transfer + completion sem.  The auto epilogue barrier is
deleted; SP waits the DMA completion sem.

Sharding: data-parallel over batch (8 cores x 256 rows).
"""

import numpy as np

B_TOTAL, IN_DIM, OUT_DIM = 2048, 64, 64
N_CORES = 8
B_SH = B_TOTAL // N_CORES  # 256 batch rows per core

_STATE = {}


def _fold(grid, coef, scale_base, scale_sp, mask):
    """Fold spline coefs + scales + mask into per-plane matmul weights."""
    g0 = np.float64(grid[0, 0])
    h = (np.float64(grid[0, -1]) - g0) / (grid.shape[1] - 1)
    inv_h = 1.0 / h
    ctr = 3.0 - g0 * inv_h  # t = u + ctr

    C = (mask * scale_sp)[:, None].astype(np.float64) * coef.astype(np.float64)
    C = C.reshape(OUT_DIM, IN_DIM, 8)
    st = np.array([1.0, -4.0, 6.0, -4.0, 1.0], np.float64) / 6.0
    Wm = np.zeros((12, IN_DIM, OUT_DIM), np.float64)
    for m in range(12):
        for j in range(max(0, m - 4), min(8, m + 1)):
            Wm[m] += C[:, :, j].T * st[m - j]
    A = (mask * scale_base).astype(np.float64).reshape(OUT_DIM, IN_DIM).T
    return Wm, A, float(h), float(inv_h), float(ctr)


def _host_prep(inputs, grid, coef, scale_base, scale_sp, mask):
    Wm, A, h, inv_h, ctr = _fold(grid, coef, scale_base, scale_sp, mask)

    # poly coefficients over leftified planes m in {0..5}
    L = range(6)
    a3 = np.zeros((IN_DIM, OUT_DIM)); a2 = np.zeros_like(a3)
    a1 = np.zeros_like(a3); a0 = np.zeros_like(a3)
    for m in L:
        d = ctr - m
        a3 += Wm[m]; a2 += 3 * d * Wm[m]
        a1 += 3 * d * d * Wm[m]; a0 += d ** 3 * Wm[m]
    bias = a0.sum(axis=0)  # (64,) f32 per-o constant

    # weight tile f16 (128, 8*64); blocks in matmul emission order:
    #  0: X2-linear [a1; 0]   1: silu A (64p)   2: SQ [a2; 0]
    #  3: CU [a3; 0]          4: chain A [-W2; -W3]  5: chain B [-W4; -W5]
    #  6: chain C [W6; W7]    7: chain D [W8; W9]
    wh = np.zeros((128, 8 * OUT_DIM), np.float64)
    wh[0:64, 0 * 64:1 * 64] = a1
    wh[0:64, 1 * 64:2 * 64] = A
    wh[0:64, 2 * 64:3 * 64] = a2
    wh[0:64, 3 * 64:4 * 64] = a3
    for k, (m0, sgn) in enumerate(((2, -1.0), (4, -1.0), (6, 1.0), (8, 1.0))):
        wh[0:64, (4 + k) * 64:(5 + k) * 64] = sgn * Wm[m0]
        wh[64:128, (4 + k) * 64:(5 + k) * 64] = sgn * Wm[m0 + 1]

    # input tile per core: [u; u-1]
    x = inputs.astype(np.float64)
    u_full = (x * inv_h).T  # (64, 2048)
    xs = []
    for c in range(N_CORES):
        u = u_full[:, c * B_SH:(c + 1) * B_SH]
        x2 = np.zeros((128, B_SH), np.float64)
        x2[0:64] = u
        x2[64:128] = u - 1.0
        xs.append(np.ascontiguousarray(x2.astype(np.float16)))

    return (xs, np.ascontiguousarray(wh.astype(np.float16)),
            np.ascontiguousarray(bias[:, None].astype(np.float32)),
            h, ctr)


def _build_nc(h=0.4, ctr=5.5):
    import concourse.bass as bass
    import concourse.bacc as bacc
    import concourse.mybir as mybir
    import concourse.tile as tile

    f32 = mybir.dt.float32
    f16 = mybir.dt.float16
    i16 = mybir.dt.int16
    AF = mybir.ActivationFunctionType
    ALU = mybir.AluOpType

    nc = bacc.Bacc("TRN2", target_bir_lowering=False, debug=False,
                   num_devices=N_CORES)
    xt = nc.dram_tensor("xt", [128, B_SH], f16, kind="ExternalInput")
    whd = nc.dram_tensor("wh", [128, 8 * OUT_DIM], f16, kind="ExternalInput")
    bsd = nc.dram_tensor("bs", [OUT_DIM, 1], f32, kind="ExternalInput")
    # 128 scatter tokens (ucode processes round_up(num_idxs,128)); rows
    # 64..127 receive garbage from the unwritten O top half - host ignores.
    out = nc.dram_tensor("out", [128, B_SH], f16, kind="ExternalOutput")

    # chain immediates on the [u; u-1] tile
    cA, cB = 2 - ctr, 4 - ctr            # min-chains: min(u - c, 0)
    iC, iD = ctr - 6, ctr - 8            # max-chains: max(u + i, 0)

    # const APs for the ACT Square biases (registered like Bacc's init
    # consts: memset pre-barrier on the otherwise-idle Pool engine)
    for v in (float(-cA), float(-cB), float(iC)):
        t = nc.alloc_sbuf_tensor(f"const-float32-{v}", [128, 1],
                                 mybir.dt.float32)
        nc.gpsimd.memset(t.ap(), v)
        nc.const_aps.aps[(mybir.dt.float32, v)] = t.ap()

    with tile.TileContext(nc) as tc:
        with tc.tile_pool(name="c", bufs=1) as cp, \
             tc.tile_pool(name="ps", bufs=1, space=bass.MemorySpace.PSUM) as pp:
            X2 = cp.tile([128, B_SH], f16)
            WH = cp.tile([128, 8 * OUT_DIM], f16)
            BS = cp.tile([OUT_DIM, 1], f32)
            nc.sync.dma_start(X2[:], xt[:])
            nc.sync.dma_start(WH[:], whd[:])
            nc.scalar.dma_start(BS[:], bsd[:])

            psum = pp.tile([OUT_DIM, B_SH], f32)

            # scatter-out bookkeeping: identity idxs via iota (the prep is
            # emitted after the O writes so its deferred data deps land on
            # the trigger; with no sem waits it still runs early on Pool)
            IDX = cp.tile([128, 8], i16)
            nc.gpsimd.iota(IDX[:], [[16, 8]], channel_multiplier=1)
            O = cp.tile([128, 1, B_SH], f16)
            dma_sem = nc.alloc_semaphore("out_dma")
            # prep emitted EARLY (descriptor gen runs on the idle Pool engine
            # at ~1900-2950, far from the trigger); because O is unwritten at
            # emission the deferred data deps are empty, so the trigger below
            # carries a manual sync dep on the copy op.
            nc.gpsimd.dma_scatter_add(
                out[:], O[:], IDX[:], num_idxs=128, num_idxs_reg=128,
                elem_size=B_SH, prepare_only=True, sem=dma_sem)

            # ACT: parallel squares for chains A,B (Square(x + c) masked by
            # the relu/min factor later), then silu.
            SpA = cp.tile([128, B_SH], f16)
            nc.scalar.activation(SpA[:], X2[:], AF.Square, bias=float(-cA))
            SpB = cp.tile([128, B_SH], f16)
            nc.scalar.activation(SpB[:], X2[:], AF.Square, bias=float(-cB))
            SpC = cp.tile([128, B_SH], f16)
            nc.scalar.activation(SpC[:], X2[:], AF.Square, bias=float(iC))
            SIL = cp.tile([64, B_SH], f16)
            nc.scalar.activation(SIL[:], X2[0:64, :], AF.Silu, scale=h)

            # DVE: TS shifts; TS_D first so the Pool chain starts early
            RD = cp.tile([128, B_SH], f16)
            nc.vector.tensor_scalar(RD[:], X2[:], iD, 0.0, ALU.add, ALU.max)
            RA = cp.tile([128, B_SH], f16)
            nc.vector.tensor_scalar(RA[:], X2[:], cA, 0.0, ALU.subtract, ALU.min)
            RB = cp.tile([128, B_SH], f16)
            nc.vector.tensor_scalar(RB[:], X2[:], cB, 0.0, ALU.subtract, ALU.min)
            RC = cp.tile([128, B_SH], f16)
            nc.vector.tensor_scalar(RC[:], X2[:], iC, 0.0, ALU.add, ALU.max)

            # Pool: square of chain D (after TS_D), cube of D later
            SD = cp.tile([128, B_SH], f16)
            nc.gpsimd.tensor_mul(SD[:], RD[:], RD[:])

            # DVE: poly square+cube first (only need X2), then chains;
            # nosync-chained so the tile scheduler keeps this order
            HB = B_SH // 2
            SQ = cp.tile([128, B_SH], f16)
            i1 = nc.vector.tensor_mul(SQ[:], X2[:], X2[:])
            CA = cp.tile([128, B_SH], f16)
            i2 = nc.vector.tensor_mul(CA[:], SpA[:], RA[:])
            CU = cp.tile([128, B_SH], f16)
            i3 = nc.vector.tensor_mul(CU[:], SQ[:], X2[:])
            # chain D cube split: Pool half + DVE half (frees Pool early
            # so the scatter prep never gates the trigger)
            CD = cp.tile([128, B_SH], f16)
            nc.gpsimd.tensor_mul(CD[:, 0:HB], SD[:, 0:HB], RD[:, 0:HB])
            i4 = nc.vector.tensor_mul(CD[:, HB:], SD[:, HB:], RD[:, HB:])
            CB = cp.tile([128, B_SH], f16)
            i5 = nc.vector.tensor_mul(CB[:], SpB[:], RB[:])
            CC = cp.tile([128, B_SH], f16)
            i6 = nc.vector.tensor_mul(CC[:], SpC[:], RC[:])
            from concourse.instruction_name_ordered_set import \
                InstructionNameOrderedSet
            for a, b in zip((i1, i2, i3, i4, i5), (i2, i3, i4, i5, i6)):
                _s = InstructionNameOrderedSet()
                _s.add(a.ins.name)
                b.ins.add_nosync_dependencies_from(_s)

            # matmuls in expected readiness order
            nc.tensor.matmul(psum[:], WH[:, 0 * 64:1 * 64], X2[:],
                             start=True, stop=False)
            nc.tensor.matmul(psum[:], WH[:, 2 * 64:3 * 64], SQ[:],
                             start=False, stop=False)
            nc.tensor.matmul(psum[:], WH[:, 3 * 64:4 * 64], CU[:],
                             start=False, stop=False)
            nc.tensor.matmul(psum[:], WH[:, 4 * 64:5 * 64], CA[:],
                             start=False, stop=False)
            nc.tensor.matmul(psum[:], WH[:, 7 * 64:8 * 64], CD[:],
                             start=False, stop=False)
            nc.tensor.matmul(psum[:], WH[:, 5 * 64:6 * 64], CB[:],
                             start=False, stop=False)
            nc.tensor.matmul(psum[:], WH[:, 6 * 64:7 * 64], CC[:],
                             start=False, stop=False)
            nc.tensor.matmul(psum[:], WH[0:64, 1 * 64:2 * 64], SIL[:],
                             start=False, stop=True)

            # PSUM -> SBUF f16 with per-o bias (single full-width ACT op)
            cpy = nc.scalar.activation(O[0:64, 0, :], psum[:],
                                       AF.Identity, bias=BS[:])

            trig = nc.gpsimd.trigger_dma(count=None)
            _d = InstructionNameOrderedSet()
            _d.add(cpy.ins.name)
            trig.ins.add_sync_dependencies_from(_d)
            nc.sync.wait_ge(dma_sem, 16)

    _hoist_input_dmas(nc, mybir)

    # Emit the activation-table load (silu_and_others, set 18) before the
    # init barrier so it overlaps the input-DMA latency.
    atl = mybir.InstLoadActFuncSet(name=nc.get_next_instruction_name(),
                                   act_func_set_id=18, ins=[], outs=[])
    atl.engine = mybir.EngineType.Activation
    main = nc.main_func.blocks[0]
    pos = next(k for k, i in enumerate(main.instructions)
               if isinstance(i, mybir.InstDrain)
               and i.engine == mybir.EngineType.Activation)
    main.instructions.insert(pos, atl)

    # Delete the auto epilogue barrier entirely: the only end-of-program
    # obligation is the out-DMA completion sem, waited on SP inside the
    # tile block.
    epi = nc.main_func.blocks[2]
    del epi.instructions[:]

    nc.compile()
    return nc


def _hoist_input_dmas(nc, mybir):
    """Move the X2/WH input DMAs ahead of the init all-engine barrier."""
    main = nc.main_func.blocks[0]
    tileblk = nc.main_func.blocks[1]

    sp_dmas = [i for i in tileblk.instructions
               if isinstance(i, mybir.InstDMACopy)
               and i.engine == mybir.EngineType.SP][:2]   # xt, wh loads

    sp_drain = next(k for k, i in enumerate(main.instructions)
                    if isinstance(i, mybir.InstDrain)
                    and i.engine == mybir.EngineType.SP)
    for insn in reversed(sp_dmas):
        tileblk.instructions.remove(insn)
        main.instructions.insert(sp_drain, insn)


def kernel(**inputs):
    x = np.asarray(inputs["inputs"], dtype=np.float32)
    grid = np.asarray(inputs["grid"], dtype=np.float32)
    coef = np.asarray(inputs["coef"], dtype=np.float32)
    scale_base = np.asarray(inputs["scale_base"], dtype=np.float32)
    scale_sp = np.asarray(inputs["scale_sp"], dtype=np.float32)
    mask = np.asarray(inputs["mask"], dtype=np.float32)

    xs, wh, bs, h, ctr = _host_prep(x, grid, coef, scale_base, scale_sp, mask)

    key = ("nc", h, ctr)
    if key not in _STATE:
        _STATE[key] = _build_nc(h, ctr)
    nc = _STATE[key]

    from concourse.bass_utils import run_bass_kernel_spmd

    in_maps = [{"xt": xs[c], "wh": wh, "bs": bs} for c in range(N_CORES)]

    res = run_bass_kernel_spmd(nc, in_maps, list(range(N_CORES)),
                               **_STATE.get("run_kwargs", {}))
    _STATE["last_results"] = res
    out_t = np.concatenate([res.results[c]["out"][0:OUT_DIM]
                            for c in range(N_CORES)], axis=1)  # (64, 2048)
    return np.ascontiguousarray(out_t.T).astype(np.float32)
